# revision 11
# baseline (speedup 1.0000x reference)
"""Trainium2 Bass kernel for the AttentionLoop module.

Reference computation (S=2048, B=32, D=1024, E=1024):
    h = tanh(einsum('sbd,ed->sbe', dec + enc, W_fc))
    scores = einsum('sbe,e->bs', h, score_w[:,0])
    attn = softmax(scores, axis=1)          # over seq
    out = einsum('bs,sbd->bd', attn, enc)   # (B, D)

Data-parallel over batch across 8 NeuronCores (4 batches/core), core-local.

Per-core kernel, hybrid-precision, h in [e-part, s-free] orientation:
  - pass-1 matmuls use W chunks as stationary, enc chunks as moving:
    out tile = [128 e, 512 s] PSUM accumulated over d-chunks. The first
    2*NPAIR d-chunks run as fp8(e4m3) DoubleRow matmuls (2 K-rows/cycle),
    the rest as bf16 matmuls. fp8 operands are pre-scaled on the host
    (enc*4, W*32 -> psum 128x); the bf16 W copy is pre-scaled by 128 so
    all partial products share one scale, folded out in the tanh evac.
  - decoder bias decW[b,e] varies along partitions here, so it rides the
    ScalarE activation as a per-partition bias: h = tanh(ps/128 + decw).
    decW itself is computed on-device in [e-part, b] orientation by tiny
    N=4 matmuls (W chunk stationary, dec moving), interleaved into batch
    0's e-chunk loop so it never stalls the tensor queue on the wb DMA.
  - scores: per e-chunk, matmul with stationary sw chunk [128,1] (M=1)
    over moving h [128, 512 s], PSUM-accumulated across the 8 e-chunks.
    Score matmuls are emitted one e-chunk behind pass-1 (software
    pipelining) so the tensor queue never waits on the ScalarE evacs.
  - softmax skips max-subtraction (scores are O(1)); Exp activation with
    accum_out gives the denominator partials for free.
  - pass-2 out[b] = p @ enc: for batches 0..2 it runs on the DVE (one
    scalar_tensor_tensor with accum_out per d-chunk over the resident
    bf16 enc tiles), hidden under the next batch's pass-1. The last
    batch's pass-2 would be fully exposed (~25us of DVE + broadcast), so
    it runs on the TensorE instead: p is transposed to columns via
    [128,128] stationary slices of the broadcast p times a one-hot
    moving vector (K=1 matmuls return zeros on TRN2 hardware), then 32
    matmuls against a [s-part, d] bf16 enc copy of that one batch.
"""

import numpy as np

S, B, D, E = 2048, 32, 1024, 1024
NCORES = 8
BLOC = B // NCORES          # 4 batches per core
P = 128                     # partitions
DC = D // P                 # 8 d-chunks
EC = E // P                 # 8 e-chunks
SB = 512                    # moving free dim (PSUM bank)
NSBLK = S // SB             # 4 s-blocks per batch
NSC = S // P                # 16 s-chunks per batch

NPAIR = 3                   # d-chunk pairs done in fp8 DoubleRow
F8CH = 2 * NPAIR            # d-chunks covered by fp8
ENC_SCALE = 4.0             # host pre-scale on fp8 enc
W_SCALE = 32.0              # host pre-scale on fp8 W
PSUM_SCALE = ENC_SCALE * W_SCALE   # bf16 W copy is scaled by this too

_compiled = None


def _build_program():
    import concourse.bacc as bacc
    import concourse.mybir as mybir
    import concourse.tile as tile

    f32 = mybir.dt.float32
    bf16 = mybir.dt.bfloat16
    f8 = mybir.dt.float8e4
    AF = mybir.ActivationFunctionType
    DR = mybir.MatmulPerfMode.DoubleRow

    nc = bacc.Bacc("TRN2", target_bir_lowering=False, debug=False,
                   num_devices=NCORES)

    et8_d = nc.declare_dram_parameter("et8", [F8CH * P, BLOC, S], f8,
                                      isOutput=False) if F8CH else None
    etb_d = nc.declare_dram_parameter("etb", [D, BLOC, S], bf16, isOutput=False)
    w8_d = nc.declare_dram_parameter("w8", [F8CH * P, E], f8,
                                     isOutput=False) if F8CH else None
    wb_d = nc.declare_dram_parameter("wb", [D, E], bf16, isOutput=False)
    decb_d = nc.declare_dram_parameter("decb", [D, BLOC], bf16, isOutput=False)
    swc_d = nc.declare_dram_parameter("swc", [P, EC], bf16, isOutput=False)
    en3_d = nc.declare_dram_parameter("en3", [S, D], bf16, isOutput=False)
    out_d = nc.declare_dram_parameter("out", [BLOC, D], f32, isOutput=True)

    with tile.TileContext(nc) as tc:
        with tc.tile_pool(name="const", bufs=1) as const, \
             tc.tile_pool(name="et8", bufs=2) as et8_pool, \
             tc.tile_pool(name="etb", bufs=2) as etb_pool, \
             tc.tile_pool(name="h", bufs=3) as h_pool, \
             tc.tile_pool(name="pbc", bufs=2) as pbc_pool, \
             tc.tile_pool(name="scr", bufs=2) as scr_pool, \
             tc.tile_pool(name="misc", bufs=2) as misc, \
             tc.tile_pool(name="ph", bufs=3, space="PSUM") as ph_pool, \
             tc.tile_pool(name="pdw", bufs=1, space="PSUM") as pdw_pool, \
             tc.tile_pool(name="psc", bufs=4, space="PSUM") as psc_pool:

            etb_r = etb_d.ap().rearrange("(dc p) b s -> p dc b s", p=P)
            wb_r = wb_d.ap().rearrange("(dc p) e -> p dc e", p=P)
            decb_r = decb_d.ap().rearrange("(dc p) b -> p dc b", p=P)
            en3_r = en3_d.ap().rearrange("(sc p) d -> p sc d", p=P)
            out_r = out_d.ap().rearrange("b (dc p) -> p b dc", p=P)
            if F8CH:
                et8_r = et8_d.ap().rearrange("(c p) b s -> p c b s", p=P)
                w8_r = w8_d.ap().rearrange("(c p) e -> p c e", p=P)

            # ---- DMA priority order: tiny consts, fp8 W, batch-0 pass-1
            # tensors, then bf16 W (first needed by batch-0 evacs), then
            # batch-0 pass-2-only chunks.
            decb_sb = const.tile([P, DC, BLOC], bf16)
            nc.sync.dma_start(decb_sb[:], decb_r)
            swc_sb = const.tile([P, EC], bf16)
            nc.sync.dma_start(swc_sb[:], swc_d.ap())
            if F8CH:
                w8_sb = const.tile([P, F8CH, E], f8)
                for c in range(F8CH):
                    nc.sync.dma_start(w8_sb[:, c, :], w8_r[:, c, :])

            etb0 = etb_pool.tile([P, DC, S], bf16, tag="etb", name="etb0")
            if F8CH:
                et80 = et8_pool.tile([P, F8CH, S], f8, tag="et8", name="et80")
                for c in range(F8CH):
                    nc.sync.dma_start(et80[:, c, :], et8_r[:, c, 0, :])
            for dc in range(F8CH, DC):
                nc.sync.dma_start(etb0[:, dc, :], etb_r[:, dc, 0, :])
            # wb sliced by e-chunk: decW(ec) and the bf16 pass-1 matmuls
            # consume wb in ec order, so batch-0 only waits on 256KB.
            wb_sb = const.tile([P, DC, E], bf16)
            wb_r2 = wb_d.ap().rearrange("(dc p) (ec q) -> p dc ec q", p=P, q=P)
            for ec in range(EC):
                nc.sync.dma_start(wb_sb[:, :, ec * P:(ec + 1) * P],
                                  wb_r2[:, :, ec, :])
            for dc in range(0, F8CH):
                nc.sync.dma_start(etb0[:, dc, :], etb_r[:, dc, 0, :])

            decw_col = const.tile([P, EC, BLOC], f32)
            e0 = const.tile([P, 1], bf16)
            nc.vector.memset(e0[:], 0.0)
            nc.vector.memset(e0[0:1, :], 1.0)

            for b in range(BLOC):
                last = (b == BLOC - 1)
                if b == 0:
                    etb, et8 = etb0, (et80 if F8CH else None)
                else:
                    etb = etb_pool.tile([P, DC, S], bf16, tag="etb",
                                        name=f"etb{b}")
                    if F8CH:
                        et8 = et8_pool.tile([P, F8CH, S], f8, tag="et8",
                                            name=f"et8{b}")
                        for c in range(F8CH):
                            nc.sync.dma_start(et8[:, c, :], et8_r[:, c, b, :])
                    for dc in range(F8CH, DC):
                        nc.sync.dma_start(etb[:, dc, :], etb_r[:, dc, b, :])
                    for dc in range(0, F8CH):
                        nc.sync.dma_start(etb[:, dc, :], etb_r[:, dc, b, :])
                if last:
                    en3_sb = const.tile([P, NSC, D], bf16)
                    for sc in range(NSC):
                        nc.sync.dma_start(en3_sb[:, sc, :], en3_r[:, sc, :])

                sc_ps = [psc_pool.tile([1, SB], f32, tag="sc", name=f"sc{j}")
                         for j in range(NSBLK)]
                h_prev = None

                for ec in range(EC):
                    ecs = slice(ec * P, (ec + 1) * P)
                    h_ec = h_pool.tile([P, NSBLK, SB], bf16, tag="h",
                                       name=f"h{ec}")
                    ph_tiles = []
                    for sblk in range(NSBLK):
                        ss = slice(sblk * SB, (sblk + 1) * SB)
                        if b == 0 and sblk == NSBLK - 1:
                            # decW for this e-chunk: tiny N=4 matmuls that
                            # trail the wb DMA; slotted before the last
                            # s-block so the batch-0 tensor queue never
                            # blocks on wb, and its ScalarE Copy lands
                            # before the evacs below that read the bias.
                            pdw = pdw_pool.tile([P, BLOC], f32, tag="pdw",
                                                name=f"pdw{ec}")
                            for dc in range(DC):
                                nc.tensor.matmul(
                                    pdw[:], wb_sb[:, dc, ecs],
                                    decb_sb[:, dc, :],
                                    start=(dc == 0), stop=(dc == DC - 1))
                            nc.scalar.activation(decw_col[:, ec, :], pdw[:],
                                                 AF.Copy,
                                                 scale=1.0 / PSUM_SCALE)
                        ph = ph_pool.tile([P, SB], f32, tag="ph",
                                          name=f"ph{sblk}")
                        ph_tiles.append(ph)
                        for pr in range(NPAIR):
                            nc.tensor.matmul(
                                ph[:],
                                w8_sb[:, 2 * pr:2 * pr + 2, ecs],
                                et8[:, 2 * pr:2 * pr + 2, ss],
                                start=(pr == 0),
                                stop=(F8CH == DC and pr == NPAIR - 1),
                                perf_mode=DR)
                        for dc in range(F8CH, DC):
                            nc.tensor.matmul(
                                ph[:], wb_sb[:, dc, ecs], etb[:, dc, ss],
                                start=(NPAIR == 0 and dc == F8CH),
                                stop=(dc == DC - 1))
                        if b != 0:
                            nc.scalar.activation(
                                h_ec[:, sblk, :], ph[:], AF.Tanh,
                                bias=decw_col[:, ec, b:b + 1],
                                scale=1.0 / PSUM_SCALE)
                        # pipelined score matmul for the previous e-chunk,
                        # spread between s-block groups
                        if h_prev is not None:
                            nc.tensor.matmul(
                                sc_ps[sblk][:], swc_sb[:, ec - 1:ec],
                                h_prev[:, sblk, :],
                                start=(ec - 1 == 0), stop=False)
                    if b == 0:
                        for sblk in range(NSBLK):
                            nc.scalar.activation(
                                h_ec[:, sblk, :], ph_tiles[sblk][:], AF.Tanh,
                                bias=decw_col[:, ec, b:b + 1],
                                scale=1.0 / PSUM_SCALE)
                    h_prev = h_ec
                for sblk in range(NSBLK):
                    nc.tensor.matmul(
                        sc_ps[sblk][:], swc_sb[:, EC - 1:EC],
                        h_prev[:, sblk, :],
                        start=False, stop=True)

                # ---- softmax (no max-subtraction; scores are O(1)) ----
                p_row = misc.tile([1, S], bf16, tag="p")
                lp = misc.tile([1, NSBLK], f32, tag="lp")
                for sblk in range(NSBLK):
                    nc.scalar.activation(
                        p_row[:, sblk * SB:(sblk + 1) * SB], sc_ps[sblk][:],
                        AF.Exp, accum_out=lp[:, sblk:sblk + 1])
                lt = misc.tile([1, 1], f32, tag="lt")
                nc.vector.tensor_reduce(lt[:], lp[:], mybir.AxisListType.X,
                                        mybir.AluOpType.add)
                invl = misc.tile([1, 1], f32, tag="invl")
                nc.vector.reciprocal(invl[:], lt[:])
                p_bc = pbc_pool.tile([P, S], bf16, tag="pbc")
                for sblk in range(NSBLK):
                    ss = slice(sblk * SB, (sblk + 1) * SB)
                    nc.gpsimd.partition_broadcast(p_bc[:, ss], p_row[:, ss])

                if not last:
                    # ---- pass-2 on DVE (hidden under next batch) ----
                    invl_bc = misc.tile([P, 1], f32, tag="invlbc")
                    nc.gpsimd.partition_broadcast(invl_bc[:], invl[:])
                    ocol = misc.tile([P, DC], f32, tag="ocol")
                    scr = scr_pool.tile([P, S], bf16, tag="scr")
                    for dc in range(DC):
                        nc.vector.scalar_tensor_tensor(
                            scr[:], etb[:, dc, :], 1.0, p_bc[:],
                            mybir.AluOpType.mult, mybir.AluOpType.mult,
                            accum_out=ocol[:, dc:dc + 1])
                    outf = misc.tile([P, DC], f32, tag="outf")
                    nc.scalar.activation(outf[:], ocol[:], AF.Copy,
                                         scale=invl_bc[:])
                    nc.sync.dma_start(out_r[:, b, :], outf[:])
                else:
                    # ---- pass-2 on TensorE (exposed tail, keep short) ----
                    pcol = misc.tile([P, NSC], bf16, tag="pcol")
                    for sc in range(NSC):
                        pct = psc_pool.tile([P, 1], f32, tag="sc",
                                            name=f"pct{sc}")
                        nc.tensor.matmul(
                            pct[:], p_bc[:, sc * P:(sc + 1) * P], e0[:],
                            start=True, stop=True)
                        nc.scalar.activation(pcol[:, sc:sc + 1], pct[:],
                                             AF.Copy)
                    po = [ph_pool.tile([1, SB], f32, tag="ph", name=f"po{g}")
                          for g in range(D // SB)]
                    for sc in range(NSC):
                        for g in range(D // SB):
                            nc.tensor.matmul(
                                po[g][:], pcol[:, sc:sc + 1],
                                en3_sb[:, sc, g * SB:(g + 1) * SB],
                                start=(sc == 0), stop=(sc == NSC - 1))
                    out_row = misc.tile([1, D], f32, tag="orow")
                    for g in range(D // SB):
                        nc.scalar.activation(
                            out_row[:, g * SB:(g + 1) * SB], po[g][:],
                            AF.Copy, scale=invl[:])
                    nc.sync.dma_start(out_d.ap()[b:b + 1, :], out_row[:])

    nc.compile()
    return nc


def _get_program():
    global _compiled
    if _compiled is None:
        _compiled = _build_program()
    return _compiled


def make_in_maps(encoder_states, decoder_state, W_fc, score_w):
    """Shard + lay out + cast full inputs into per-core input maps."""
    import ml_dtypes
    f8 = ml_dtypes.float8_e4m3
    bf16 = ml_dtypes.bfloat16

    enc = np.asarray(encoder_states, dtype=np.float32)
    dec = np.asarray(decoder_state, dtype=np.float32)
    wfc = np.asarray(W_fc, dtype=np.float32)
    sw = np.asarray(score_w, dtype=np.float32)

    w_t = np.ascontiguousarray(wfc.T)                      # (D, E)
    wb = (w_t * PSUM_SCALE).astype(bf16)
    swc = np.ascontiguousarray(sw[:, 0].reshape(EC, P).T.astype(bf16))
    if F8CH:
        w8 = (w_t[:F8CH * P] * W_SCALE).astype(f8)

    in_maps = []
    for i in range(NCORES):
        b0 = i * BLOC
        sl = enc[:, b0:b0 + BLOC, :]
        ett = np.ascontiguousarray(sl.transpose(2, 1, 0))  # (D, BLOC, S)
        m = {
            "etb": ett.astype(bf16),
            "wb": wb,
            "decb": np.ascontiguousarray(dec[b0:b0 + BLOC, :].T).astype(bf16),
            "swc": swc,
            "en3": np.ascontiguousarray(sl[:, BLOC - 1, :]).astype(bf16),
        }
        if F8CH:
            m["et8"] = (ett[:F8CH * P] * ENC_SCALE).astype(f8)
            m["w8"] = w8
        in_maps.append(m)
    return in_maps


def kernel(encoder_states, decoder_state, W_fc, score_w):
    from concourse.bass_utils import run_bass_kernel_spmd

    in_maps = make_in_maps(encoder_states, decoder_state, W_fc, score_w)
    nc = _get_program()
    res = run_bass_kernel_spmd(nc, in_maps, list(range(NCORES)))
    return np.concatenate([res.results[i]["out"] for i in range(NCORES)], axis=0)


# revision 14
# speedup vs baseline: 1.0832x; 1.0832x over previous
"""Trainium2 Bass kernel for the AttentionLoop module.

Reference computation (S=2048, B=32, D=1024, E=1024):
    h = tanh(einsum('sbd,ed->sbe', dec + enc, W_fc))
    scores = einsum('sbe,e->bs', h, score_w[:,0])
    attn = softmax(scores, axis=1)          # over seq
    out = einsum('bs,sbd->bd', attn, enc)   # (B, D)

Data-parallel over batch across 8 NeuronCores (4 batches/core), core-local.

Per-core kernel, hybrid-precision, h in [e-part, s-free] orientation:
  - pass-1 matmuls use W chunks as stationary, enc chunks as moving:
    out tile = [128 e, 512 s] PSUM accumulated over d-chunks. The first
    2*NPAIR d-chunks run as fp8(e4m3) DoubleRow matmuls (2 K-rows/cycle),
    the rest as bf16 matmuls. fp8 operands are pre-scaled on the host
    (enc*4, W*32 -> psum 128x); the bf16 W copy is pre-scaled by 128 so
    all partial products share one scale, folded out in the tanh evac.
  - decoder bias decW[b,e] varies along partitions here, so it rides the
    ScalarE activation as a per-partition bias: h = tanh(ps/128 + decw).
    decW itself is computed on-device in [e-part, b] orientation by tiny
    N=4 matmuls (W chunk stationary, dec moving), interleaved into batch
    0's e-chunk loop so it never stalls the tensor queue on the wb DMA.
  - scores: per e-chunk, matmul with stationary sw chunk [128,1] (M=1)
    over moving h [128, 512 s], PSUM-accumulated across the 8 e-chunks.
    Score matmuls are emitted one e-chunk behind pass-1 (software
    pipelining) so the tensor queue never waits on the ScalarE evacs.
  - softmax skips max-subtraction (scores are O(1)); Exp activation with
    accum_out gives the denominator partials for free.
  - pass-2 out[b] = p @ enc: for batches 0..2 it runs on the DVE (one
    scalar_tensor_tensor with accum_out per d-chunk over the resident
    bf16 enc tiles), hidden under the next batch's pass-1. The last
    batch's pass-2 would be fully exposed (~25us of DVE + broadcast), so
    it runs on the TensorE instead: p is transposed to columns via
    [128,128] stationary slices of the broadcast p times a one-hot
    moving vector (K=1 matmuls return zeros on TRN2 hardware), then 32
    matmuls against a [s-part, d] bf16 enc copy of that one batch.
"""

import numpy as np

S, B, D, E = 2048, 32, 1024, 1024
NCORES = 8
BLOC = B // NCORES          # 4 batches per core
P = 128                     # partitions
DC = D // P                 # 8 d-chunks
EC = E // P                 # 8 e-chunks
SB = 512                    # moving free dim (PSUM bank)
NSBLK = S // SB             # 4 s-blocks per batch
NSC = S // P                # 16 s-chunks per batch

NPAIR = 3                   # d-chunk pairs done in fp8 DoubleRow
F8CH = 2 * NPAIR            # d-chunks covered by fp8
ENC_SCALE = 4.0             # host pre-scale on fp8 enc
W_SCALE = 32.0              # host pre-scale on fp8 W
PSUM_SCALE = ENC_SCALE * W_SCALE   # bf16 W copy is scaled by this too

_compiled = None


def _build_program():
    import concourse.bacc as bacc
    import concourse.mybir as mybir
    import concourse.tile as tile

    f32 = mybir.dt.float32
    bf16 = mybir.dt.bfloat16
    f8 = mybir.dt.float8e4
    AF = mybir.ActivationFunctionType
    DR = mybir.MatmulPerfMode.DoubleRow

    nc = bacc.Bacc("TRN2", target_bir_lowering=False, debug=False,
                   num_devices=NCORES)

    et8_d = nc.declare_dram_parameter("et8", [F8CH * P, BLOC, S], f8,
                                      isOutput=False) if F8CH else None
    etb_d = nc.declare_dram_parameter("etb", [D, BLOC, S], bf16, isOutput=False)
    w8_d = nc.declare_dram_parameter("w8", [F8CH * P, E], f8,
                                     isOutput=False) if F8CH else None
    wb_d = nc.declare_dram_parameter("wb", [D, E], bf16, isOutput=False)
    decb_d = nc.declare_dram_parameter("decb", [D, BLOC], bf16, isOutput=False)
    swc_d = nc.declare_dram_parameter("swc", [P, EC], bf16, isOutput=False)
    en3_d = nc.declare_dram_parameter("en3", [S, D], bf16, isOutput=False)
    out_d = nc.declare_dram_parameter("out", [BLOC, D], f32, isOutput=True)

    with tile.TileContext(nc) as tc:
        with tc.tile_pool(name="const", bufs=1) as const, \
             tc.tile_pool(name="et8", bufs=2) as et8_pool, \
             tc.tile_pool(name="etb", bufs=2) as etb_pool, \
             tc.tile_pool(name="h", bufs=3) as h_pool, \
             tc.tile_pool(name="pbc", bufs=2) as pbc_pool, \
             tc.tile_pool(name="scr", bufs=2) as scr_pool, \
             tc.tile_pool(name="misc", bufs=2) as misc, \
             tc.tile_pool(name="ph", bufs=3, space="PSUM") as ph_pool, \
             tc.tile_pool(name="pdw", bufs=1, space="PSUM") as pdw_pool, \
             tc.tile_pool(name="psc", bufs=4, space="PSUM") as psc_pool:

            etb_r = etb_d.ap().rearrange("(dc p) b s -> p dc b s", p=P)
            wb_r = wb_d.ap().rearrange("(dc p) e -> p dc e", p=P)
            decb_r = decb_d.ap().rearrange("(dc p) b -> p dc b", p=P)
            en3_r = en3_d.ap().rearrange("(sc p) d -> p sc d", p=P)
            out_r = out_d.ap().rearrange("b (dc p) -> p b dc", p=P)
            if F8CH:
                et8_r = et8_d.ap().rearrange("(c p) b s -> p c b s", p=P)
                w8_r = w8_d.ap().rearrange("(c p) e -> p c e", p=P)

            # ---- DMA priority order: tiny consts, fp8 W, batch-0 pass-1
            # tensors, then bf16 W (first needed by batch-0 evacs), then
            # batch-0 pass-2-only chunks.
            decb_sb = const.tile([P, DC, BLOC], bf16)
            nc.sync.dma_start(decb_sb[:], decb_r)
            swc_sb = const.tile([P, EC], bf16)
            nc.sync.dma_start(swc_sb[:], swc_d.ap())
            if F8CH:
                # w8 sliced by e-chunk: pass-1 consumes it in ec order, so
                # batch-0 can start after ~0.1MB instead of the full w8.
                w8_sb = const.tile([P, F8CH, E], f8)
                w8_r2 = w8_d.ap().rearrange("(c p) (ec q) -> p c ec q",
                                            p=P, q=P)
                for ec in range(EC):
                    nc.sync.dma_start(w8_sb[:, :, ec * P:(ec + 1) * P],
                                      w8_r2[:, :, ec, :])

            # batch-0 enc DMAs split into s-halves so the first e-chunk's
            # matmuls start after half the data is in.
            etb0 = etb_pool.tile([P, DC, S], bf16, tag="etb", name="etb0")
            if F8CH:
                et80 = et8_pool.tile([P, F8CH, S], f8, tag="et8", name="et80")
            for half in range(2):
                hs = slice(half * (S // 2), (half + 1) * (S // 2))
                if F8CH:
                    for c in range(F8CH):
                        nc.sync.dma_start(et80[:, c, hs], et8_r[:, c, 0, hs])
                for dc in range(F8CH, DC):
                    nc.sync.dma_start(etb0[:, dc, hs], etb_r[:, dc, 0, hs])
            # wb sliced by e-chunk: decW(ec) and the bf16 pass-1 matmuls
            # consume wb in ec order, so batch-0 only waits on 256KB.
            wb_sb = const.tile([P, DC, E], bf16)
            wb_r2 = wb_d.ap().rearrange("(dc p) (ec q) -> p dc ec q", p=P, q=P)
            for ec in range(EC):
                nc.sync.dma_start(wb_sb[:, :, ec * P:(ec + 1) * P],
                                  wb_r2[:, :, ec, :])
            for dc in range(0, F8CH):
                nc.sync.dma_start(etb0[:, dc, :], etb_r[:, dc, 0, :])

            decw_col = const.tile([P, EC, BLOC], f32)
            e0 = const.tile([P, 1], bf16)
            nc.vector.memset(e0[:], 0.0)
            nc.vector.memset(e0[0:1, :], 1.0)

            for b in range(BLOC):
                last = (b == BLOC - 1)
                if b == 0:
                    etb, et8 = etb0, (et80 if F8CH else None)
                else:
                    etb = etb_pool.tile([P, DC, S], bf16, tag="etb",
                                        name=f"etb{b}")
                    if F8CH:
                        et8 = et8_pool.tile([P, F8CH, S], f8, tag="et8",
                                            name=f"et8{b}")
                        for c in range(F8CH):
                            nc.sync.dma_start(et8[:, c, :], et8_r[:, c, b, :])
                    for dc in range(F8CH, DC):
                        nc.sync.dma_start(etb[:, dc, :], etb_r[:, dc, b, :])
                    for dc in range(0, F8CH):
                        nc.sync.dma_start(etb[:, dc, :], etb_r[:, dc, b, :])
                if last:
                    en3_sb = const.tile([P, NSC, D], bf16)
                    for sc in range(NSC):
                        nc.sync.dma_start(en3_sb[:, sc, :], en3_r[:, sc, :])

                sc_ps = [psc_pool.tile([1, SB], f32, tag="sc", name=f"sc{j}")
                         for j in range(NSBLK)]
                h_prev = None

                for ec in range(EC):
                    ecs = slice(ec * P, (ec + 1) * P)
                    h_ec = h_pool.tile([P, NSBLK, SB], bf16, tag="h",
                                       name=f"h{ec}")
                    ph_tiles = []
                    for sblk in range(NSBLK):
                        ss = slice(sblk * SB, (sblk + 1) * SB)
                        if b == 0 and sblk == NSBLK - 1:
                            # decW for this e-chunk: tiny N=4 matmuls that
                            # trail the wb DMA; slotted before the last
                            # s-block so the batch-0 tensor queue never
                            # blocks on wb, and its ScalarE Copy lands
                            # before the evacs below that read the bias.
                            pdw = pdw_pool.tile([P, BLOC], f32, tag="pdw",
                                                name=f"pdw{ec}")
                            for dc in range(DC):
                                nc.tensor.matmul(
                                    pdw[:], wb_sb[:, dc, ecs],
                                    decb_sb[:, dc, :],
                                    start=(dc == 0), stop=(dc == DC - 1))
                            nc.scalar.activation(decw_col[:, ec, :], pdw[:],
                                                 AF.Copy,
                                                 scale=1.0 / PSUM_SCALE)
                        ph = ph_pool.tile([P, SB], f32, tag="ph",
                                          name=f"ph{sblk}")
                        ph_tiles.append(ph)
                        for pr in range(NPAIR):
                            nc.tensor.matmul(
                                ph[:],
                                w8_sb[:, 2 * pr:2 * pr + 2, ecs],
                                et8[:, 2 * pr:2 * pr + 2, ss],
                                start=(pr == 0),
                                stop=(F8CH == DC and pr == NPAIR - 1),
                                perf_mode=DR)
                        for dc in range(F8CH, DC):
                            nc.tensor.matmul(
                                ph[:], wb_sb[:, dc, ecs], etb[:, dc, ss],
                                start=(NPAIR == 0 and dc == F8CH),
                                stop=(dc == DC - 1))
                        if b != 0:
                            nc.scalar.activation(
                                h_ec[:, sblk, :], ph[:], AF.Tanh,
                                bias=decw_col[:, ec, b:b + 1],
                                scale=1.0 / PSUM_SCALE)
                    if b == 0:
                        for sblk in range(NSBLK):
                            nc.scalar.activation(
                                h_ec[:, sblk, :], ph_tiles[sblk][:], AF.Tanh,
                                bias=decw_col[:, ec, b:b + 1],
                                scale=1.0 / PSUM_SCALE)
                    # pipelined score matmuls for the previous e-chunk
                    # (clustered: interleaving them between s-block groups
                    # breaks the stationary-weight pipelining)
                    if h_prev is not None:
                        for sblk in range(NSBLK):
                            nc.tensor.matmul(
                                sc_ps[sblk][:], swc_sb[:, ec - 1:ec],
                                h_prev[:, sblk, :],
                                start=(ec - 1 == 0), stop=False)
                    h_prev = h_ec
                if not last:
                    for sblk in range(NSBLK):
                        nc.tensor.matmul(
                            sc_ps[sblk][:], swc_sb[:, EC - 1:EC],
                            h_prev[:, sblk, :],
                            start=False, stop=True)

                    # ---- softmax (no max-subtraction; scores are O(1)) ---
                    p_row = misc.tile([1, S], bf16, tag="p")
                    lp = misc.tile([1, NSBLK], f32, tag="lp")
                    for sblk in range(NSBLK):
                        nc.scalar.activation(
                            p_row[:, sblk * SB:(sblk + 1) * SB],
                            sc_ps[sblk][:],
                            AF.Exp, accum_out=lp[:, sblk:sblk + 1])
                    lt = misc.tile([1, 1], f32, tag="lt")
                    nc.vector.tensor_reduce(lt[:], lp[:],
                                            mybir.AxisListType.X,
                                            mybir.AluOpType.add)
                    invl = misc.tile([1, 1], f32, tag="invl")
                    nc.vector.reciprocal(invl[:], lt[:])
                    p_bc = pbc_pool.tile([P, S], bf16, tag="pbc")
                    for sblk in range(NSBLK):
                        ss = slice(sblk * SB, (sblk + 1) * SB)
                        nc.gpsimd.partition_broadcast(p_bc[:, ss],
                                                      p_row[:, ss])

                    # ---- pass-2 on DVE (hidden under next batch) ----
                    invl_bc = misc.tile([P, 1], f32, tag="invlbc")
                    nc.gpsimd.partition_broadcast(invl_bc[:], invl[:])
                    ocol = misc.tile([P, DC], f32, tag="ocol")
                    scr = scr_pool.tile([P, S], bf16, tag="scr")
                    for dc in range(DC):
                        nc.vector.scalar_tensor_tensor(
                            scr[:], etb[:, dc, :], 1.0, p_bc[:],
                            mybir.AluOpType.mult, mybir.AluOpType.mult,
                            accum_out=ocol[:, dc:dc + 1])
                    outf = misc.tile([P, DC], f32, tag="outf")
                    nc.scalar.activation(outf[:], ocol[:], AF.Copy,
                                         scale=invl_bc[:])
                    nc.sync.dma_start(out_r[:, b, :], outf[:])
                else:
                    # ---- exposed tail: per-s-block pipeline of score-stop,
                    # exp, broadcast, transpose and pass-2 matmuls, so only
                    # the last s-block's chain is serial at the end ----
                    p_row = misc.tile([1, S], bf16, tag="p")
                    lp = misc.tile([1, NSBLK], f32, tag="lp")
                    p_bc = pbc_pool.tile([P, S], bf16, tag="pbc")
                    pcol = misc.tile([P, NSC], bf16, tag="pcol")
                    pct_all = ph_pool.tile([P, NSC], f32, tag="ph",
                                           name="pct_all")
                    po = [ph_pool.tile([1, SB], f32, tag="ph", name=f"po{g}")
                          for g in range(D // SB)]
                    npsb = NSC // NSBLK  # p-columns per s-block
                    for sblk in range(NSBLK):
                        ss = slice(sblk * SB, (sblk + 1) * SB)
                        nc.tensor.matmul(
                            sc_ps[sblk][:], swc_sb[:, EC - 1:EC],
                            h_prev[:, sblk, :],
                            start=False, stop=True)
                        nc.scalar.activation(
                            p_row[:, ss], sc_ps[sblk][:],
                            AF.Exp, accum_out=lp[:, sblk:sblk + 1])
                        nc.gpsimd.partition_broadcast(p_bc[:, ss],
                                                      p_row[:, ss])
                        for k in range(npsb):
                            sc = sblk * npsb + k
                            nc.tensor.matmul(
                                pct_all[:, sc:sc + 1],
                                p_bc[:, sc * P:(sc + 1) * P], e0[:],
                                start=True, stop=True,
                                skip_group_check=True)
                            nc.scalar.activation(pcol[:, sc:sc + 1],
                                                 pct_all[:, sc:sc + 1],
                                                 AF.Copy)
                        for k in range(npsb):
                            sc = sblk * npsb + k
                            for g in range(D // SB):
                                nc.tensor.matmul(
                                    po[g][:], pcol[:, sc:sc + 1],
                                    en3_sb[:, sc, g * SB:(g + 1) * SB],
                                    start=(sc == 0), stop=(sc == NSC - 1))
                    lt = misc.tile([1, 1], f32, tag="lt")
                    nc.vector.tensor_reduce(lt[:], lp[:],
                                            mybir.AxisListType.X,
                                            mybir.AluOpType.add)
                    invl = misc.tile([1, 1], f32, tag="invl")
                    nc.vector.reciprocal(invl[:], lt[:])
                    out_row = misc.tile([1, D], f32, tag="orow")
                    for g in range(D // SB):
                        nc.scalar.activation(
                            out_row[:, g * SB:(g + 1) * SB], po[g][:],
                            AF.Copy, scale=invl[:])
                    nc.sync.dma_start(out_d.ap()[b:b + 1, :], out_row[:])

    nc.compile()
    return nc


def _get_program():
    global _compiled
    if _compiled is None:
        _compiled = _build_program()
    return _compiled


def make_in_maps(encoder_states, decoder_state, W_fc, score_w):
    """Shard + lay out + cast full inputs into per-core input maps."""
    import ml_dtypes
    f8 = ml_dtypes.float8_e4m3
    bf16 = ml_dtypes.bfloat16

    enc = np.asarray(encoder_states, dtype=np.float32)
    dec = np.asarray(decoder_state, dtype=np.float32)
    wfc = np.asarray(W_fc, dtype=np.float32)
    sw = np.asarray(score_w, dtype=np.float32)

    w_t = np.ascontiguousarray(wfc.T)                      # (D, E)
    wb = (w_t * PSUM_SCALE).astype(bf16)
    swc = np.ascontiguousarray(sw[:, 0].reshape(EC, P).T.astype(bf16))
    if F8CH:
        w8 = (w_t[:F8CH * P] * W_SCALE).astype(f8)

    in_maps = []
    for i in range(NCORES):
        b0 = i * BLOC
        sl = enc[:, b0:b0 + BLOC, :]
        ett = np.ascontiguousarray(sl.transpose(2, 1, 0))  # (D, BLOC, S)
        m = {
            "etb": ett.astype(bf16),
            "wb": wb,
            "decb": np.ascontiguousarray(dec[b0:b0 + BLOC, :].T).astype(bf16),
            "swc": swc,
            "en3": np.ascontiguousarray(sl[:, BLOC - 1, :]).astype(bf16),
        }
        if F8CH:
            m["et8"] = (ett[:F8CH * P] * ENC_SCALE).astype(f8)
            m["w8"] = w8
        in_maps.append(m)
    return in_maps


def kernel(encoder_states, decoder_state, W_fc, score_w):
    from concourse.bass_utils import run_bass_kernel_spmd

    in_maps = make_in_maps(encoder_states, decoder_state, W_fc, score_w)
    nc = _get_program()
    res = run_bass_kernel_spmd(nc, in_maps, list(range(NCORES)))
    return np.concatenate([res.results[i]["out"] for i in range(NCORES)], axis=0)
